# revision 3
# baseline (speedup 1.0000x reference)
"""EfficientAttention (linear attention) Trainium2 Bass kernel.

Computes, per batch b:
    q_n = softmax(q[b], axis=-1)        # over feature dim D=64
    k_n = softmax(k[b], axis=-1)
    ctx = k_n^T @ v[b]                  # [D, D]
    out[b] = q_n @ ctx                  # [N, D]

Sharding: batch dim (32) split across 8 cores, 4 batches per core.

Design notes (per core):
- fp16 I/O: the host casts q/k/v to fp16 and the kernel stores fp16
  outputs (cast back to fp32 on host). Halves HBM traffic vs fp32 —
  this kernel is HBM-bound (32 MB @ ~358 GB/s/core ≈ 89 us floor).
  Input quantization adds ~1e-3 rel err, well inside the 2e-2 gate.
- DMA: 1 MB loads/stores, >=8 KB contiguous per partition. k loads
  trigger via sync (SP HWDGE ring), v loads + o stores via scalar
  (ACT HWDGE ring) to balance the two rings.
- K/V pass (natural layout, tokens on partitions): exp(k) on ACT,
  row-sums + reciprocal on DVE, normalize on Pool, PE accumulates
  ctx[64,64] over N via 128 K=128 matmuls per batch.
- ctx epilogue: block-diagonal stacked ctxa [128, 130] fp16
  (rows 0:64 = [ctx | 1 | 0], rows 64:128 = [0 | ctx | 1]) so one K=128
  matmul handles two tokens (even/odd) packed on one partition pair
  group. (Matmuls with alternating row groups writing one PSUM bank
  lock up the device - found by bisection.)
- Q pass via DMA-transpose: q[b] viewed as [8192 row-pairs, 128] is
  transposed by the DMA xbar during the load itself -> qT [128, pairs]
  fp16 (partitions = 2x64 stacked features of even/odd rows). ACT exp
  -> eq. Matmul chunk c uses the strided stationary eq[:, c::32]
  (pairs c+32j -> output partition j holds 64 consecutive rows so the
  fp16 store is 8 KB contiguous per partition). Each K=128 matmul
  against ctxa yields [128, 130] = even vals|sum | odd vals|sum; DVE
  reciprocal + multiply (3 chunks per PSUM bank) -> fp16 output.
- Batch b's q-pass is interleaved with batch b+1's k/v-pass to keep
  all engines dense.
"""

import numpy as np

import concourse.bass as bass
import concourse.mybir as mybir
import concourse.tile as tile
from concourse import bacc
from concourse.bass_utils import run_bass_kernel_spmd

B, N, D = 32, 16384, 64
NCORES = 8
BPC = B // NCORES  # batches per core
LOAD = 8192  # rows per DMA block (1 MB fp16)
LT = LOAD // 128  # row-tile slots per load (64)
NBLK = N // LOAD  # load blocks per batch (2)
QP = LOAD // 2  # row-pairs per q block (4096)
CH = 32  # matmul chunks per q block (128 pairs each)
F32 = mybir.dt.float32
F16 = mybir.dt.float16
EXP = mybir.ActivationFunctionType.Exp


def build_bass():
    nc = bacc.Bacc("TRN2", target_bir_lowering=False, debug=False)
    q = nc.dram_tensor("q", [BPC, N, D], F16, kind="ExternalInput").ap()
    k = nc.dram_tensor("k", [BPC, N, D], F16, kind="ExternalInput").ap()
    v = nc.dram_tensor("v", [BPC, N, D], F16, kind="ExternalInput").ap()
    o = nc.dram_tensor("o", [BPC, N, D], F16, kind="ExternalOutput").ap()

    def blk(t, b, n0):
        return t[b, n0 : n0 + LOAD, :].rearrange("(p t) d -> p t d", p=128)

    with tile.TileContext(nc) as tc:
        with (
            tc.tile_pool(name="io", bufs=2) as io,
            tc.tile_pool(name="work", bufs=3) as work,
            tc.tile_pool(name="ctxp", bufs=2) as ctxp,
            tc.tile_pool(name="ps_o", bufs=5, space="PSUM") as ps_o,
            tc.tile_pool(name="ps_c", bufs=2, space="PSUM") as ps_c,
        ):
            ctx_ps = {}

            def emit_kv_block(b, i):
                n0 = i * LOAD
                k_sb = io.tile([128, LT, 64], F16, tag="k_sb", bufs=3)
                v_sb = io.tile([128, LT, 64], F16, tag="v_sb", bufs=3)
                nc.sync.dma_start(out=k_sb, in_=blk(k, b, n0))
                nc.scalar.dma_start(out=v_sb, in_=blk(v, b, n0))
                ek = work.tile([128, LT, 64], F16, tag="ek")
                nc.scalar.activation(ek, k_sb, EXP)
                ks = work.tile([128, LT, 1], F32, tag="ks")
                nc.vector.reduce_sum(out=ks, in_=ek, axis=mybir.AxisListType.X)
                ksr = work.tile([128, LT, 1], F32, tag="ksr")
                nc.vector.reciprocal(ksr, ks)
                ekn = work.tile([128, LT, 64], F16, tag="ekn", bufs=4)
                nc.gpsimd.tensor_mul(ekn, ek, ksr[:].to_broadcast((128, LT, 64)))
                for t in range(LT):
                    nc.tensor.matmul(
                        ctx_ps[b],
                        ekn[:, t, :],
                        v_sb[:, t, :],
                        start=(i == 0 and t == 0),
                        stop=(i == NBLK - 1 and t == LT - 1),
                    )

            def emit_ctx_epilogue(b):
                ctxa = ctxp.tile([128, 130], F16, tag="ctxa")
                nc.vector.memset(ctxa, 0.0)
                nc.vector.tensor_copy(ctxa[0:64, 0:64], ctx_ps[b])
                nc.vector.memset(ctxa[0:64, 64:65], 1.0)
                nc.scalar.dma_start(out=ctxa[64:128, 65:130], in_=ctxa[0:64, 0:65])
                return ctxa

            def load_qT_block(b, i):
                # DMA-xbar-transposed load: q rows as [4096 pairs, 128] ->
                # qT [128, 4096]: partition w*64+d = feature d of parity-w
                # row, free = pair index.
                qT = io.tile([128, QP], F16, tag="qT", bufs=3, name="qT")
                src = q[b, i * LOAD : (i + 1) * LOAD, :].rearrange(
                    "(r t) d -> r (t d)", t=2
                )
                nc.sync.dma_start_transpose(qT, src)
                return qT

            def emit_q_block(b, i, ctxa, qT=None, split_store=False):
                n0 = i * LOAD
                if qT is None:
                    qT = load_qT_block(b, i)
                eq = work.tile([128, QP], F16, tag="eq", bufs=2)
                nc.scalar.activation(eq, qT, EXP)
                eqa = eq[:]
                pd_eq = eqa.ap[0]
                out_sb = io.tile([128, CH, 2, 64], F16, tag="out_sb", bufs=3)
                # bank groups of 3 chunks (3 x 132 fp32 <= 512/bank)
                groups = [(c0, min(3, CH - c0)) for c0 in range(0, CH, 3)]
                for c0, nch in groups:
                    o_ps = ps_o.tile([128, 3, 132], F32, tag="o_ps")
                    for s in range(nch):
                        lhsT = bass.AP(
                            tensor=eqa.tensor,
                            offset=eqa.offset + c0 + s,
                            ap=[pd_eq, [CH, 128]],
                        )
                        nc.tensor.matmul(
                            o_ps[:, s, 0:130],
                            lhsT,
                            ctxa,
                            start=True,
                            stop=True,
                        )
                    opb = o_ps[:]
                    pdim = opb.ap[0]
                    r_sb = work.tile([128, 3, 2, 1], F32, tag="r_sb")
                    rs_ap = bass.AP(
                        tensor=opb.tensor,
                        offset=opb.offset + 64,
                        ap=[pdim, [132, nch], [65, 2], [1, 1]],
                    )
                    nc.vector.reciprocal(r_sb[:, 0:nch], rs_ap)
                    vals_ap = bass.AP(
                        tensor=opb.tensor,
                        offset=opb.offset,
                        ap=[pdim, [132, nch], [65, 2], [1, 64]],
                    )
                    nc.vector.tensor_mul(
                        out_sb[:, c0 : c0 + nch, :, :],
                        vals_ap,
                        r_sb[:, 0:nch].to_broadcast((128, nch, 2, 64)),
                    )
                # partition j holds rows n0+64j .. n0+64j+63 (8 KB contig)
                dst = o[b, n0 : n0 + LOAD, :].rearrange(
                    "(j c w) d -> j c w d", j=128, w=2
                )
                if split_store:
                    for c0 in range(0, CH, 8):
                        nc.scalar.dma_start(
                            out=dst[:, c0 : c0 + 8],
                            in_=out_sb[:, c0 : c0 + 8],
                        )
                else:
                    nc.scalar.dma_start(out=dst, in_=out_sb)

            # software-pipelined schedule: q-pass(b) interleaved with kv(b+1)
            ctx_ps[0] = ps_c.tile([64, 64], F32, tag="ctx_ps", name="ctx_ps")
            q_pre = [load_qT_block(0, 0), load_qT_block(0, 1)]
            for i in range(NBLK):
                emit_kv_block(0, i)
            ctxa = emit_ctx_epilogue(0)
            for b in range(BPC):
                if b + 1 < BPC:
                    ctx_ps[b + 1] = ps_c.tile([64, 64], F32, tag="ctx_ps", name="ctx_ps")
                nxt = None
                for i in range(NBLK):
                    # kv(b+1) first so its ctx completes before q(b) drains;
                    # epilogue right after the last kv block
                    if b + 1 < BPC:
                        emit_kv_block(b + 1, i)
                        if i == NBLK - 1:
                            nxt = emit_ctx_epilogue(b + 1)
                    last = b == BPC - 1 and i == NBLK - 1
                    emit_q_block(
                        b, i, ctxa,
                        qT=q_pre.pop(0) if (b == 0 and q_pre) else None,
                        split_store=last,
                    )
                if nxt is not None:
                    ctxa = nxt

    nc.compile()
    return nc


_NC_CACHE = None


def kernel(q: np.ndarray, k: np.ndarray, v: np.ndarray) -> np.ndarray:
    global _NC_CACHE
    if _NC_CACHE is None:
        _NC_CACHE = build_bass()
    nc = _NC_CACHE
    q = np.ascontiguousarray(np.asarray(q), dtype=np.float16)
    k = np.ascontiguousarray(np.asarray(k), dtype=np.float16)
    v = np.ascontiguousarray(np.asarray(v), dtype=np.float16)
    in_maps = [
        {
            "q": q[i * BPC : (i + 1) * BPC],
            "k": k[i * BPC : (i + 1) * BPC],
            "v": v[i * BPC : (i + 1) * BPC],
        }
        for i in range(NCORES)
    ]
    res = run_bass_kernel_spmd(nc, in_maps, core_ids=list(range(NCORES)))
    return np.concatenate(
        [res.results[i]["o"] for i in range(NCORES)], axis=0
    ).astype(np.float32)
